# revision 40
# baseline (speedup 1.0000x reference)
"""Bass/Trainium2 kernel for Kimi-style MLA attention (nn_KimiMLAAttention).

Strategy (8 NeuronCores, tensor-parallel over heads):
  - 16 heads -> 2 heads per core. Each core computes q-projection for its 2
    heads, a 1/8 sequence-shard of the compressed-kv projection + rmsnorm
    (AllGathered across cores into the full shared latent), per-head
    k-embed / v-unembed from the latent, causal attention in a TRANSPOSED
    score layout (scores^T[s, l]) so no on-chip transposes are needed, and
    a partial o_proj against its 2-head slice of Wo.
  - Host sums the 8 partial o_proj outputs (the "all-reduce after o_proj").

Profiling on this part shows the PE processes ~0.74 ns per moving-dim row
regardless of operand dtype (core clock is util-throttled), so runtime is
dominated by total matmul moving-rows. The kernel therefore:
  - runs all matmuls with bf16 operands (halves LDWEIGHTS + SBUF);
  - shards the latent projection over the sequence (8x fewer rows) and
    AllGathers the normalized bf16 latent through DRAM bounce buffers,
    overlapped with the q-projection;
  - accumulates softmax denominators on the vector engine (esum) instead
    of per-tile PE colsum matmuls;
  - trims the moving width of causal-diagonal score/AV matmuls to the
    valid l-range.
fp32 is kept where it matters: PSUM accumulation, rmsnorm scale rows, the
softmax denominator sum + reciprocal. The o_proj partial output is DMAed
out as bf16 and summed across cores in fp32 on the host.
"""

from contextlib import ExitStack

import ml_dtypes
import numpy as np

import concourse.bass as bass
import concourse.tile as tile
from concourse import bass_isa, mybir
from concourse.bass import ds, ts
from concourse.bass_utils import run_bass_kernel_spmd

F32 = mybir.dt.float32
F32R = mybir.dt.float32r
BF16 = mybir.dt.bfloat16
AF = mybir.ActivationFunctionType
BF = ml_dtypes.bfloat16


def _patch_tile_tail_drain():
    """walrus's CoreV3 codegen rejects the TileContext tail drain when it
    carries >1 sem waits ("Too many sync wait commands"). Split the waits
    across multiple single-wait drain instructions on the sync engine."""
    if getattr(tile.TileContext, "_tail_drain_patched", False):
        return
    from concourse.vector_clock import ScopedClock

    def _drain_and_barrier(self, tick_clock, wait_clock):
        nc = self.nc
        drain_inst = nc.sync.drain()
        wait_clock.add_sem_waits(
            drain_inst.ins, ScopedClock({None: tick_clock.global_clock})
        )
        inst = drain_inst.ins
        si = inst.sync_info
        if si is not None and si.on_wait is not None and len(si.on_wait) > 1:
            waits = list(si.on_wait)
            upd = list(si.on_update) if si.on_update else []
            inst.sync_info = mybir.SyncInfo(on_wait=waits[:1], on_update=[])
            for i, w in enumerate(waits[1:]):
                extra = nc.sync.drain()
                last = i == len(waits) - 2
                extra.ins.sync_info = mybir.SyncInfo(
                    on_wait=[w], on_update=upd if last else []
                )
        nc.all_engine_barrier()
        assert self.sems is not None
        popped = nc._tile_sem_poison_stack.pop()
        assert popped is self._sem_poison
        nc.clear_and_free_semaphores(list(self.sems.allocated().values()))
        nc.all_engine_barrier()

    tile.TileContext._drain_and_barrier = _drain_and_barrier
    tile.TileContext._tail_drain_patched = True


_patch_tile_tail_drain()


def _split_excess_waits(nc, max_waits=1):
    """walrus's per-instruction sync-wait slots are tiny on this compiler
    build; hoist excess sem waits onto same-engine NoOp carriers placed
    immediately before the instruction (waits fire earlier in the same
    engine stream, so ordering semantics are preserved)."""
    for f in nc.m.functions:
        for bb in f.blocks:
            insts = bb.instructions
            if not any(
                i.sync_info is not None
                and i.sync_info.on_wait
                and len(i.sync_info.on_wait) > max_waits
                for i in insts
            ):
                continue
            out = []
            for inst in insts:
                si = inst.sync_info
                if si is not None and si.on_wait and len(si.on_wait) > max_waits:
                    waits = list(si.on_wait)
                    for w in waits[:-max_waits]:
                        nop = mybir.InstNoOp(
                            name=nc.get_next_instruction_name(), ins=[], outs=[]
                        )
                        nop.engine = inst.engine
                        nop.sync_info = mybir.SyncInfo(on_wait=[w], on_update=[])
                        out.append(nop)
                    inst.sync_info = mybir.SyncInfo(
                        on_wait=waits[-max_waits:],
                        on_update=list(si.on_update) if si.on_update else [],
                    )
                out.append(inst)
            bb.instructions = out


B, L, HID = 1, 2048, 2048
H = 16
NOPE, ROPE, VDIM, LORA = 128, 64, 128, 512
QDIM = NOPE + ROPE
EPS = 1e-5
SCALE = QDIM**-0.5
NCORES = 8
HPC = H // NCORES  # 2 heads per core

LCH = 512  # moving-operand chunk (max N per matmul / PSUM bank)
NJ = L // LCH  # 4 l-chunks
NK = HID // 128  # 16 contraction tiles for projections
NS = L // 128  # 16 s(key)-tiles
NLAT = LORA // 128  # 4 latent partition tiles
WCOLS = 1024  # fused projection weight columns (64 zero-pad cols after k_pe
# so the latent shard runs 5 uniform 128-col matmuls — no 64-col PE config)
SW = L // NCORES  # 256: per-core latent sequence shard
CKV = LORA + ROPE  # 576 rows per shard in the allgather buffer


def _build_nc():
    nc = bass.Bass(num_devices=NCORES)
    xT = nc.dram_tensor("xT", [HID, L], BF16, kind="ExternalInput")
    # per-core latent x-shard, host-permuted to [p, k, c] so one contiguous
    # DMA lands all 16 contraction tiles side by side in SBUF
    xTs = nc.dram_tensor("xTs", [128, NK * SW], BF16, kind="ExternalInput")
    wqkv = nc.dram_tensor("wqkv", [HID, WCOLS], BF16, kind="ExternalInput")
    we = nc.dram_tensor("we", [HPC, LORA, NOPE], BF16, kind="ExternalInput")
    wu = nc.dram_tensor("wu", [LORA, HPC * VDIM], BF16, kind="ExternalInput")
    wo0 = nc.dram_tensor("wo0", [VDIM, HID], BF16, kind="ExternalInput")
    wo1 = nc.dram_tensor("wo1", [VDIM, HID], BF16, kind="ExternalInput")
    mbig = nc.dram_tensor("mbig", [128, 896], BF16, kind="ExternalInput")
    ones_col_d = nc.dram_tensor("ones_col_d", [128, 1], F32R, kind="ExternalInput")
    ones_row_d = nc.dram_tensor("ones_row_d", [1, 128], F32R, kind="ExternalInput")
    y = nc.dram_tensor("y", [L, HID], BF16, kind="ExternalOutput")

    mm = nc.tensor.matmul

    with tile.TileContext(nc) as tc, ExitStack() as ctx:
        persist = ctx.enter_context(tc.tile_pool(name="persist", bufs=1))
        dram = ctx.enter_context(tc.tile_pool(name="dram", bufs=1, space="DRAM"))
        # q^T nope per head [128, L]; ropes packed [h0 rope p0:64 | h1 rope p64:128]
        qn = [persist.tile([128, L], BF16, name=f"qn{h}", tag=f"qn{h}") for h in range(HPC)]
        qr = persist.tile([128, L], BF16, name="qr", tag="qr")
        # k_pe^T zero-padded per head: kpeZ[0] has k_pe on partitions 0:64 (h0's
        # rope rows in qr), kpeZ[1] on 64:128. Keeps every score matmul on the
        # full 128x128 PE config — 64-contraction matmuls force a PE tile
        # reconfig that costs ~200 ns on this part.
        kpeZ = [persist.tile([128, L], BF16, name=f"kpeZ{h}", tag=f"kpeZ{h}") for h in range(HPC)]
        # v for both heads: [s-in-tile, si*256 + h*128 + vd]
        vsb = persist.tile([128, NS * HPC * VDIM], BF16, name="vsb", tag="vsb")
        kT = [persist.tile([128, L], BF16, name=f"kT{h}", tag=f"kT{h}") for h in range(HPC)]
        outT = [persist.tile([128, L], BF16, name=f"outT{h}", tag=f"outT{h}") for h in range(HPC)]
        mask_sb = persist.tile([128, 896], BF16, name="mask_sb", tag="mask_sb")
        ones_col = persist.tile([128, 1], F32R, name="ones_col", tag="ones_col")
        ones_row = persist.tile([1, 128], F32R, name="ones_row", tag="ones_row")
        eps_col = persist.tile([128, 1], F32, name="eps_col", tag="eps_col")
        nc.sync.dma_start(out=mask_sb, in_=mbig[:, :])
        nc.sync.dma_start(out=ones_col, in_=ones_col_d[:, :])
        nc.sync.dma_start(out=ones_row, in_=ones_row_d[:, :])
        nc.vector.memset(eps_col, EPS)
        nc.vector.memset(kpeZ[0][64:128, :], 0.0)
        nc.vector.memset(kpeZ[1][0:64, :], 0.0)

        # AllGather bounce buffers: [4x128 latent | 64 k_pe] rows x SW cols
        cc_in = dram.tile([CKV, SW], BF16, name="cc_in", tag="cc_in")
        # Shared addr space: HBM-HBM AllGather into Local DRAM takes a slow
        # staging path (and the runtime warns about it)
        cc_out = dram.tile([NCORES * CKV, SW], BF16, name="cc_out", tag="cc_out", addr_space="Shared")

        with tc.tile_pool(name="latpool", bufs=1) as latpool:
            latB = [latpool.tile([128, L], BF16, name=f"latB{i}", tag=f"latB{i}") for i in range(NLAT)]

            with tc.tile_pool(name="wq_pool", bufs=1) as wqp:
                # weight tiles are DMAed just-in-time inside the P0a k-loop so
                # the first matmul doesn't sit behind 16 queued weight DMAs
                w_sb = [wqp.tile([128, WCOLS], BF16, name=f"w{k}", tag=f"w{k}") for k in range(NK)]

                # ---- P0a: this core's latent shard ckv^T = Wkv.T @ xTs + rmsnorm ----
                with (
                    tc.tile_pool(name="shardp", bufs=1) as shp,
                    tc.tile_pool(name="xs_pool", bufs=3) as xsp,
                    tc.tile_pool(name="ps0n", bufs=1, space="PSUM") as ppn,
                ):
                    NM = NLAT + 1  # 4 latent tiles + [k_pe | zero-pad]
                    latS = [shp.tile([128, SW], F32R, name=f"lS{i}", tag=f"lS{i}") for i in range(NLAT)]
                    kpeS = shp.tile([64, SW], BF16, name="kpeS", tag="kpeS")
                    xss = xsp.tile([128, NK * SW], BF16, name="xss", tag="xss")
                    nc.scalar.dma_start(out=xss, in_=xTs[:, :])
                    with tc.tile_pool(name="ps0a", bufs=1, space="PSUM") as ppa:
                        psa = [ppa.tile([128, SW], F32, name=f"pa{i}", tag=f"pa{i}") for i in range(NM)]
                        for k in range(NK):
                            nc.sync.dma_start(out=w_sb[k], in_=wqkv[ts(k, 128), :])
                            for i in range(NM):
                                mm(psa[i], (w_sb[k][:, ds(384 + 128 * i, 128)]), (xss[:, ds(k * SW, SW)]),
                                   start=(k == 0), stop=(k == NK - 1))
                        for i in range(NLAT):
                            nc.vector.tensor_copy(latS[i], psa[i])
                        nc.vector.tensor_copy(kpeS, psa[NLAT][0:64, :])
                    # rmsnorm: scale = rsqrt(mean(c^2) + eps)
                    ssq = ppn.tile([1, SW], F32, name="ssq", tag="ssq")
                    for i in range(NLAT):
                        sq = shp.tile([128, SW], F32R, name="sq", tag="sq")
                        nc.vector.tensor_mul(sq, latS[i], latS[i])
                        mm(ssq, (ones_col), (sq), start=(i == 0), stop=(i == NLAT - 1))
                    sqrt_row = shp.tile([1, SW], F32, name="sqrt_row", tag="row1")
                    nc.scalar.activation(sqrt_row, ssq, AF.Sqrt, bias=eps_col[0:1, :], scale=1.0 / LORA)
                    scale_row = shp.tile([1, SW], F32R, name="scale_row", tag="row2")
                    with nc.allow_low_precision(reason="fp32r row for broadcast matmul"):
                        nc.vector.reciprocal(scale_row, sqrt_row)
                    bc = ppn.tile([128, SW], F32, name="bc", tag="bc")

                    def _latent_tail():
                        # deferred into P0b so the bc matmul (which waits on the
                        # reciprocal) doesn't stall queued PE work (FIFO queue)
                        mm(bc, (ones_row), (scale_row), start=True, stop=True)
                        latSB = [shp.tile([128, SW], BF16, name=f"lB{i}", tag=f"lB{i}") for i in range(NLAT)]
                        with nc.allow_low_precision(reason="bf16 latent feed for PE"):
                            for i in range(NLAT):
                                nc.vector.tensor_mul(latSB[i], latS[i], bc)
                        # ship the shard (gpsimd queue so the sync queue stays free)
                        for i in range(NLAT):
                            nc.gpsimd.dma_start(out=cc_in[ts(i, 128), :], in_=latSB[i])
                        nc.gpsimd.dma_start(out=cc_in[ds(LORA, ROPE), :], in_=kpeS)
                        nc.gpsimd.collective_compute(
                            "AllGather",
                            mybir.AluOpType.bypass,
                            replica_groups=[list(range(NCORES))],
                            ins=[cc_in[:, :].opt()],
                            outs=[cc_out[:, :].opt()],
                        )
                        # unpack gathered latent/kpe; spread the 48 DMA
                        # triggers over three engine queues (each ~0.7us of
                        # engine time) so latB isn't gated on one serial queue
                        qs = [nc.gpsimd, nc.scalar, nc.sync]
                        qi = 0
                        for c in range(NCORES):
                            cw = ds(c * SW, SW)
                            for i in range(NLAT):
                                qs[qi % 3].dma_start(out=latB[i][:, cw], in_=cc_out[ds(c * CKV + i * 128, 128), :])
                                qi += 1
                            qs[qi % 3].dma_start(out=kpeZ[0][0:64, cw], in_=cc_out[ds(c * CKV + LORA, ROPE), :])
                            qi += 1
                            qs[qi % 3].dma_start(out=kpeZ[1][64:128, cw], in_=cc_out[ds(c * CKV + LORA, ROPE), :])
                            qi += 1

                    # ---- P0b: q projections qT = Wq.T @ xT-chunks (overlaps AllGather) ----
                    with (
                        tc.tile_pool(name="x_pool", bufs=3) as xp,
                        tc.tile_pool(name="ps0", bufs=1, space="PSUM") as pp0,
                    ):
                        MS = [(0, 128), (128, 128), (256, 128)]  # h0 nope, h1 nope, ropes
                        for j in range(NJ):
                            jc = ds(j * LCH, LCH)
                            pss = [pp0.tile([128, LCH], F32, name=f"pm{m}", tag=f"pm{m}") for m in range(3)]
                            for k in range(NK):
                                xt = xp.tile([128, LCH], BF16, name="xt", tag="xt")
                                nc.sync.dma_start(out=xt, in_=xT[ts(k, 128), jc])
                                for m, (c0, cw) in enumerate(MS):
                                    mm(pss[m][:cw, :], (w_sb[k][:, ds(c0, cw)]), (xt),
                                       start=(k == 0), stop=(k == NK - 1))
                                if j == 0 and k == 2:
                                    _latent_tail()
                            nc.vector.tensor_copy(qn[0][:, jc], pss[0])
                            nc.vector.tensor_copy(qn[1][:, jc], pss[1])
                            nc.vector.tensor_copy(qr[:, jc], pss[2])

            # ---- P2: per-head k^T = We'.T @ latB, v = latB.T @ Wu' ----
            with (
                tc.tile_pool(name="wep", bufs=1) as wep,
                tc.tile_pool(name="ps2", bufs=2, space="PSUM") as pp2,
            ):
                # weight loads on the scalar queue: the sync queue still has
                # P0b's x-tile triggers ahead of it at this point
                we_sb = []
                for h in range(HPC):
                    row = []
                    for i in range(NLAT):
                        t = wep.tile([128, NOPE], BF16, name=f"we{h}{i}", tag=f"we{h}{i}")
                        nc.scalar.dma_start(out=t, in_=we[h, ts(i, 128), :])
                        row.append(t)
                    we_sb.append(row)
                wu_sb = []
                for i in range(NLAT):
                    t = wep.tile([128, HPC * VDIM], BF16, name=f"wu{i}", tag=f"wu{i}")
                    nc.scalar.dma_start(out=t, in_=wu[ts(i, 128), :])
                    wu_sb.append(t)
                for si in range(NS):
                    pv = pp2.tile([128, HPC * VDIM], F32, name="pv", tag="pv")
                    for i in range(NLAT):
                        mm(pv, (latB[i][:, ts(si, 128)]), (wu_sb[i]),
                           start=(i == 0), stop=(i == NLAT - 1))
                    nc.vector.tensor_copy(vsb[:, ds(si * HPC * VDIM, HPC * VDIM)], pv)
                for h in range(HPC):
                    for j in range(NJ):
                        jc = ds(j * LCH, LCH)
                        pk = pp2.tile([128, LCH], F32, name="pk", tag="pk")
                        for i in range(NLAT):
                            mm(pk, (we_sb[h][i]), (latB[i][:, jc]),
                               start=(i == 0), stop=(i == NLAT - 1))
                        nc.vector.tensor_copy(kT[h][:, jc], pk)

        # ---- P3: causal attention in transposed layout, per (head, l-chunk) ----
        # Diagonal-band s-tiles (d = si - 4j in 0..3) only touch l >= 128d in
        # the chunk, so their matmuls/exp run on the trimmed moving range and
        # only the leading 128-wide diagonal block needs the triangle mask.
        with (
            tc.tile_pool(name="wop", bufs=1) as wop,
            tc.tile_pool(name="epool", bufs=3) as epool,
            tc.tile_pool(name="rows", bufs=2) as rows,
            tc.tile_pool(name="esums", bufs=2) as esums,
            tc.tile_pool(name="ps3", bufs=1, space="PSUM") as pp3,
        ):
            tri = mask_sb[:, ds(384, 128)]  # tri[p, q] = 1 iff q >= p
            # The PE queue is FIFO, so the pbc broadcast matmul (which waits on
            # the ~3.3us DVE reciprocal) is deferred into the NEXT (h,j)'s
            # score-matmul stream — by the time it issues, the reciprocal is
            # long done and no PE stall occurs.
            pending_norm = None
            for h in range(HPC):
                for j in range(NJ):
                    jc = ds(j * LCH, LCH)
                    nsi = 4 * j + 4  # causal: s-tiles 0..4j+3 touch l-chunk j
                    po = pp3.tile([128, LCH], F32, name="po", tag="po", bufs=2)
                    esum = esums.tile([128, LCH], F32R, name="esum", tag="esum")
                    for si in range(nsi):
                        d = si - 4 * j
                        off = max(0, 128 * d)  # valid l starts here (trimmed)
                        w = LCH - off
                        sub = ds(j * LCH + off, w)
                        psub = ds(off, w)
                        ps = pp3.tile([128, LCH], F32, name="ps", tag="ps", bufs=2)
                        mm(ps[:, psub], (kT[h][:, ts(si, 128)]), (qn[h][:, sub]),
                           start=True, stop=False)
                        mm(ps[:, psub], (kpeZ[h][:, ts(si, 128)]), (qr[:, sub]),
                           start=False, stop=True)
                        e = epool.tile([128, LCH], BF16, name="e", tag="e")
                        nc.scalar.activation(e[:, psub], ps[:, psub], AF.Exp, scale=SCALE)
                        if d >= 0:  # mask the 128-wide diagonal block
                            nc.vector.tensor_mul(e[:, ds(off, 128)], e[:, ds(off, 128)], tri)
                        with nc.allow_low_precision(reason="fp32 esum of bf16 tiles"):
                            if si == 0:
                                nc.vector.tensor_copy(esum, e)
                            else:
                                nc.vector.tensor_add(esum[:, psub], esum[:, psub], e[:, psub])
                        mm(po[:, psub], (vsb[:, ds(si * HPC * VDIM + h * VDIM, VDIM)]), (e[:, psub]),
                           start=(si == 0), stop=(si == nsi - 1), skip_group_check=True)
                        if si == 3 and pending_norm is not None:
                            pending_norm()
                            pending_norm = None
                    pcs = pp3.tile([1, LCH], F32, name="pcs", tag="pcs", bufs=2)
                    mm(pcs, (ones_col), (esum), start=True, stop=True)
                    # reciprocal of the [1,512] denominator row on one DVE lane
                    # costs 3.3us (6.5 ns/elem/lane) and stalled the deferred
                    # broadcast matmul; bounce it through a [128,4] reshape so
                    # all 128 lanes split the work (~150 ns)
                    crow = rows.tile([1, LCH], F32, name="crow", tag="crow")
                    nc.scalar.copy(crow, pcs)
                    prsh = rows.tile([128, 4], F32, name="prsh", tag="prsh")
                    nc.gpsimd.dma_start(out=prsh, in_=crow)
                    rcp4 = rows.tile([128, 4], F32R, name="rcp4", tag="rcp4")
                    with nc.allow_low_precision(reason="fp32r row for broadcast matmul"):
                        nc.vector.reciprocal(rcp4, prsh)
                    rrow = rows.tile([1, LCH], F32R, name="rrow", tag="rrow")
                    nc.gpsimd.dma_start(out=rrow, in_=rcp4)

                    def _norm_tail(h=h, jc=jc, po=po, rrow=rrow):
                        pbc = pp3.tile([128, LCH], F32, name="pbc", tag="pbc", bufs=2)
                        mm(pbc, (ones_row), (rrow), start=True, stop=True)
                        bcs = epool.tile([128, LCH], F32, name="bcs", tag="bcs")
                        nc.vector.tensor_copy(bcs, pbc)
                        with nc.allow_low_precision(reason="bf16 outT feed for PE"):
                            nc.vector.tensor_mul(outT[h][:, jc], po, bcs)

                    pending_norm = _norm_tail
            pending_norm()
            pending_norm = None

            # ---- P4: partial o_proj y = outT.T @ Wo[2-head rows] ----
            wo_sb = []
            for hh, wsrc in enumerate([wo0, wo1]):
                t = wop.tile([128, HID], BF16, name=f"wo{hh}", tag=f"wo{hh}")
                nc.scalar.dma_start(out=t, in_=wsrc[:, :])
                wo_sb.append(t)
            for i in range(NS):
                for n in range(NJ):
                    py = pp3.tile([128, LCH], F32, name="py", tag="ps", bufs=2)
                    mm(py, (outT[0][:, ts(i, 128)]), (wo_sb[0][:, ds(n * LCH, LCH)]),
                       start=True, stop=False)
                    mm(py, (outT[1][:, ts(i, 128)]), (wo_sb[1][:, ds(n * LCH, LCH)]),
                       start=False, stop=True)
                    ysb = epool.tile([128, LCH], BF16, name="ysb", tag="ysb", bufs=3)
                    if (i * NJ + n) % 2 == 0:  # split PSUM->SBUF casts across engines
                        nc.scalar.copy(ysb, py)
                    else:
                        nc.vector.tensor_copy(ysb, py)
                    nc.sync.dma_start(out=y[ts(i, 128), ds(n * LCH, LCH)], in_=ysb)

    _split_excess_waits(nc)
    return nc


_NC_CACHE = None


def _get_nc():
    global _NC_CACHE
    if _NC_CACHE is None:
        _NC_CACHE = _build_nc()
    return _NC_CACHE


def _make_in_maps(x, Wq, Wkv_a, kv_ln_w, W_embed, W_unembed, Wo):
    xT = np.ascontiguousarray(np.asarray(x, dtype=np.float32)[0].T).astype(BF)
    Wq = np.asarray(Wq, dtype=np.float32)
    Wkv_a = np.asarray(Wkv_a, dtype=np.float32)
    kv_ln_w = np.asarray(kv_ln_w, dtype=np.float32)
    W_embed = np.asarray(W_embed, dtype=np.float32)
    W_unembed = np.asarray(W_unembed, dtype=np.float32)
    Wo = np.asarray(Wo, dtype=np.float32)

    Wq3 = Wq.reshape(HID, H, QDIM)
    # diagonal-band mask template: mbig[p, q] = 1 iff (q - 384) >= p
    q_idx = np.arange(896) - 384
    p_idx = np.arange(128)
    mbig = (q_idx[None, :] >= p_idx[:, None]).astype(BF)

    in_maps = []
    for c in range(NCORES):
        h0, h1 = HPC * c, HPC * c + 1
        wqkv = np.concatenate(
            [
                Wq3[:, h0, :NOPE],
                Wq3[:, h1, :NOPE],
                Wq3[:, h0, NOPE:],
                Wq3[:, h1, NOPE:],
                Wkv_a,
                np.zeros((HID, WCOLS - 384 - Wkv_a.shape[1]), np.float32),
            ],
            axis=1,
        )
        we = np.ascontiguousarray(W_embed[[h0, h1]] * kv_ln_w[None, :, None])
        wu = np.ascontiguousarray(
            np.concatenate([W_unembed[h0].T, W_unembed[h1].T], axis=1) * kv_ln_w[:, None]
        )
        in_maps.append(
            {
                "xT": xT,
                "xTs": np.ascontiguousarray(
                    xT[:, c * SW : (c + 1) * SW].reshape(NK, 128, SW).transpose(1, 0, 2).reshape(128, NK * SW)
                ),
                "wqkv": np.ascontiguousarray(wqkv).astype(BF),
                "we": we.astype(BF),
                "wu": wu.astype(BF),
                "wo0": np.ascontiguousarray(Wo[h0 * VDIM : (h0 + 1) * VDIM]).astype(BF),
                "wo1": np.ascontiguousarray(Wo[h1 * VDIM : (h1 + 1) * VDIM]).astype(BF),
                "mbig": mbig,
                "ones_col_d": np.ones((128, 1), np.float32),
                "ones_row_d": np.ones((1, 128), np.float32),
            }
        )
    return in_maps


def run(trace=False, tmpdir=None, **inputs):
    """Run the SPMD kernel; returns (full_output, BassKernelResults)."""
    inputs.pop("mask", None)  # causal structure is hardcoded
    nc = _get_nc()
    in_maps = _make_in_maps(**inputs)
    res = run_bass_kernel_spmd(
        nc, in_maps, core_ids=list(range(NCORES)), trace=trace, tmpdir=tmpdir
    )
    y = np.zeros((L, HID), dtype=np.float32)
    for c in range(NCORES):
        y += np.asarray(res.results[c]["y"], dtype=np.float32)
    return y.reshape(B, L, HID), res


def kernel(**inputs):
    y, _ = run(trace=False, **inputs)
    return y


# revision 41
# speedup vs baseline: 1.1114x; 1.1114x over previous
"""Bass/Trainium2 kernel for Kimi-style MLA attention (nn_KimiMLAAttention).

Strategy (8 NeuronCores, tensor-parallel over heads):
  - 16 heads -> 2 heads per core. Each core computes q-projection for its 2
    heads, a 1/8 sequence-shard of the compressed-kv projection + rmsnorm
    (AllGathered across cores into the full shared latent), per-head
    k-embed / v-unembed from the latent, causal attention in a TRANSPOSED
    score layout (scores^T[s, l]) so no on-chip transposes are needed, and
    a partial o_proj against its 2-head slice of Wo.
  - Host sums the 8 partial o_proj outputs (the "all-reduce after o_proj").

Profiling on this part shows the PE processes ~0.74 ns per moving-dim row
regardless of operand dtype (core clock is util-throttled), so runtime is
dominated by total matmul moving-rows. The kernel therefore:
  - runs all matmuls with bf16 operands (halves LDWEIGHTS + SBUF);
  - shards the latent projection over the sequence (8x fewer rows) and
    AllGathers the normalized bf16 latent through DRAM bounce buffers,
    overlapped with the q-projection;
  - accumulates softmax denominators on the vector engine (esum) instead
    of per-tile PE colsum matmuls;
  - trims the moving width of causal-diagonal score/AV matmuls to the
    valid l-range.
fp32 is kept where it matters: PSUM accumulation, rmsnorm scale rows, the
softmax denominator sum + reciprocal. The o_proj partial output is DMAed
out as bf16 and summed across cores in fp32 on the host.
"""

from contextlib import ExitStack

import ml_dtypes
import numpy as np

import concourse.bass as bass
import concourse.tile as tile
from concourse import bass_isa, mybir
from concourse.bass import ds, ts
from concourse.bass_utils import run_bass_kernel_spmd

F32 = mybir.dt.float32
F32R = mybir.dt.float32r
BF16 = mybir.dt.bfloat16
AF = mybir.ActivationFunctionType
BF = ml_dtypes.bfloat16


def _patch_tile_tail_drain():
    """walrus's CoreV3 codegen rejects the TileContext tail drain when it
    carries >1 sem waits ("Too many sync wait commands"). Split the waits
    across multiple single-wait drain instructions on the sync engine."""
    if getattr(tile.TileContext, "_tail_drain_patched", False):
        return
    from concourse.vector_clock import ScopedClock

    def _drain_and_barrier(self, tick_clock, wait_clock):
        nc = self.nc
        drain_inst = nc.sync.drain()
        wait_clock.add_sem_waits(
            drain_inst.ins, ScopedClock({None: tick_clock.global_clock})
        )
        inst = drain_inst.ins
        si = inst.sync_info
        if si is not None and si.on_wait is not None and len(si.on_wait) > 1:
            waits = list(si.on_wait)
            upd = list(si.on_update) if si.on_update else []
            inst.sync_info = mybir.SyncInfo(on_wait=waits[:1], on_update=[])
            for i, w in enumerate(waits[1:]):
                extra = nc.sync.drain()
                last = i == len(waits) - 2
                extra.ins.sync_info = mybir.SyncInfo(
                    on_wait=[w], on_update=upd if last else []
                )
        nc.all_engine_barrier()
        assert self.sems is not None
        popped = nc._tile_sem_poison_stack.pop()
        assert popped is self._sem_poison
        nc.clear_and_free_semaphores(list(self.sems.allocated().values()))
        nc.all_engine_barrier()

    tile.TileContext._drain_and_barrier = _drain_and_barrier
    tile.TileContext._tail_drain_patched = True


_patch_tile_tail_drain()


def _split_excess_waits(nc, max_waits=1):
    """walrus's per-instruction sync-wait slots are tiny on this compiler
    build; hoist excess sem waits onto same-engine NoOp carriers placed
    immediately before the instruction (waits fire earlier in the same
    engine stream, so ordering semantics are preserved)."""
    for f in nc.m.functions:
        for bb in f.blocks:
            insts = bb.instructions
            if not any(
                i.sync_info is not None
                and i.sync_info.on_wait
                and len(i.sync_info.on_wait) > max_waits
                for i in insts
            ):
                continue
            out = []
            for inst in insts:
                si = inst.sync_info
                if si is not None and si.on_wait and len(si.on_wait) > max_waits:
                    waits = list(si.on_wait)
                    for w in waits[:-max_waits]:
                        nop = mybir.InstNoOp(
                            name=nc.get_next_instruction_name(), ins=[], outs=[]
                        )
                        nop.engine = inst.engine
                        nop.sync_info = mybir.SyncInfo(on_wait=[w], on_update=[])
                        out.append(nop)
                    inst.sync_info = mybir.SyncInfo(
                        on_wait=waits[-max_waits:],
                        on_update=list(si.on_update) if si.on_update else [],
                    )
                out.append(inst)
            bb.instructions = out


B, L, HID = 1, 2048, 2048
H = 16
NOPE, ROPE, VDIM, LORA = 128, 64, 128, 512
QDIM = NOPE + ROPE
EPS = 1e-5
SCALE = QDIM**-0.5
NCORES = 8
HPC = H // NCORES  # 2 heads per core

LCH = 512  # moving-operand chunk (max N per matmul / PSUM bank)
NJ = L // LCH  # 4 l-chunks
NK = HID // 128  # 16 contraction tiles for projections
NS = L // 128  # 16 s(key)-tiles
NLAT = LORA // 128  # 4 latent partition tiles
WCOLS = 1024  # fused projection weight columns (64 zero-pad cols after k_pe
# so the latent shard runs 5 uniform 128-col matmuls — no 64-col PE config)
SW = L // NCORES  # 256: per-core latent sequence shard
CKV = LORA + ROPE  # 576 rows per shard in the allgather buffer


def _build_nc():
    nc = bass.Bass(num_devices=NCORES)
    xT = nc.dram_tensor("xT", [HID, L], BF16, kind="ExternalInput")
    # per-core latent x-shard, host-permuted to [p, k, c] so one contiguous
    # DMA lands all 16 contraction tiles side by side in SBUF
    xTs = nc.dram_tensor("xTs", [128, NK * SW], BF16, kind="ExternalInput")
    wqkv = nc.dram_tensor("wqkv", [HID, WCOLS], BF16, kind="ExternalInput")
    we = nc.dram_tensor("we", [HPC, LORA, NOPE], BF16, kind="ExternalInput")
    wu = nc.dram_tensor("wu", [LORA, HPC * VDIM], BF16, kind="ExternalInput")
    wo0 = nc.dram_tensor("wo0", [VDIM, HID], BF16, kind="ExternalInput")
    wo1 = nc.dram_tensor("wo1", [VDIM, HID], BF16, kind="ExternalInput")
    mbig = nc.dram_tensor("mbig", [128, 896], BF16, kind="ExternalInput")
    ones_col_d = nc.dram_tensor("ones_col_d", [128, 1], F32R, kind="ExternalInput")
    ones_row_d = nc.dram_tensor("ones_row_d", [1, 128], F32R, kind="ExternalInput")
    y = nc.dram_tensor("y", [L, HID], BF16, kind="ExternalOutput")

    mm = nc.tensor.matmul

    with tile.TileContext(nc) as tc, ExitStack() as ctx:
        persist = ctx.enter_context(tc.tile_pool(name="persist", bufs=1))
        dram = ctx.enter_context(tc.tile_pool(name="dram", bufs=1, space="DRAM"))
        # q^T nope per head [128, L]; ropes packed [h0 rope p0:64 | h1 rope p64:128]
        qn = [persist.tile([128, L], BF16, name=f"qn{h}", tag=f"qn{h}") for h in range(HPC)]
        qr = persist.tile([128, L], BF16, name="qr", tag="qr")
        # k_pe^T zero-padded per head: kpeZ[0] has k_pe on partitions 0:64 (h0's
        # rope rows in qr), kpeZ[1] on 64:128. Keeps every score matmul on the
        # full 128x128 PE config — 64-contraction matmuls force a PE tile
        # reconfig that costs ~200 ns on this part.
        kpeZ = [persist.tile([128, L], BF16, name=f"kpeZ{h}", tag=f"kpeZ{h}") for h in range(HPC)]
        # v for both heads: [s-in-tile, si*256 + h*128 + vd]
        vsb = persist.tile([128, NS * HPC * VDIM], BF16, name="vsb", tag="vsb")
        kT = [persist.tile([128, L], BF16, name=f"kT{h}", tag=f"kT{h}") for h in range(HPC)]
        outT = [persist.tile([128, L], BF16, name=f"outT{h}", tag=f"outT{h}") for h in range(HPC)]
        mask_sb = persist.tile([128, 896], BF16, name="mask_sb", tag="mask_sb")
        ones_col = persist.tile([128, 1], F32R, name="ones_col", tag="ones_col")
        ones_row = persist.tile([1, 128], F32R, name="ones_row", tag="ones_row")
        eps_col = persist.tile([128, 1], F32, name="eps_col", tag="eps_col")
        nc.sync.dma_start(out=mask_sb, in_=mbig[:, :])
        nc.sync.dma_start(out=ones_col, in_=ones_col_d[:, :])
        nc.sync.dma_start(out=ones_row, in_=ones_row_d[:, :])
        nc.vector.memset(eps_col, EPS)
        nc.vector.memset(kpeZ[0][64:128, :], 0.0)
        nc.vector.memset(kpeZ[1][0:64, :], 0.0)

        # AllGather bounce buffers: [4x128 latent | 64 k_pe] rows x SW cols
        cc_in = dram.tile([CKV, SW], BF16, name="cc_in", tag="cc_in")
        # Shared addr space: HBM-HBM AllGather into Local DRAM takes a slow
        # staging path (and the runtime warns about it)
        cc_out = dram.tile([NCORES * CKV, SW], BF16, name="cc_out", tag="cc_out", addr_space="Shared")

        with tc.tile_pool(name="latpool", bufs=1) as latpool:
            latB = [latpool.tile([128, L], BF16, name=f"latB{i}", tag=f"latB{i}") for i in range(NLAT)]

            with tc.tile_pool(name="wq_pool", bufs=1) as wqp:
                # weight tiles are DMAed just-in-time inside the P0a k-loop so
                # the first matmul doesn't sit behind 16 queued weight DMAs
                w_sb = [wqp.tile([128, WCOLS], BF16, name=f"w{k}", tag=f"w{k}") for k in range(NK)]

                # ---- P0a: this core's latent shard ckv^T = Wkv.T @ xTs + rmsnorm ----
                with (
                    tc.tile_pool(name="shardp", bufs=1) as shp,
                    tc.tile_pool(name="xs_pool", bufs=3) as xsp,
                    tc.tile_pool(name="ps0n", bufs=1, space="PSUM") as ppn,
                ):
                    NM = NLAT + 1  # 4 latent tiles + [k_pe | zero-pad]
                    latS = [shp.tile([128, SW], F32R, name=f"lS{i}", tag=f"lS{i}") for i in range(NLAT)]
                    kpeS = shp.tile([64, SW], BF16, name="kpeS", tag="kpeS")
                    xss = xsp.tile([128, NK * SW], BF16, name="xss", tag="xss")
                    nc.scalar.dma_start(out=xss, in_=xTs[:, :])
                    with tc.tile_pool(name="ps0a", bufs=1, space="PSUM") as ppa:
                        psa = [ppa.tile([128, SW], F32, name=f"pa{i}", tag=f"pa{i}") for i in range(NM)]
                        for k in range(NK):
                            nc.sync.dma_start(out=w_sb[k], in_=wqkv[ts(k, 128), :])
                            for i in range(NM):
                                mm(psa[i], (w_sb[k][:, ds(384 + 128 * i, 128)]), (xss[:, ds(k * SW, SW)]),
                                   start=(k == 0), stop=(k == NK - 1))
                        for i in range(NLAT):
                            nc.vector.tensor_copy(latS[i], psa[i])
                        nc.vector.tensor_copy(kpeS, psa[NLAT][0:64, :])
                    # rmsnorm: scale = rsqrt(mean(c^2) + eps)
                    ssq = ppn.tile([1, SW], F32, name="ssq", tag="ssq")
                    for i in range(NLAT):
                        sq = shp.tile([128, SW], F32R, name="sq", tag="sq")
                        nc.vector.tensor_mul(sq, latS[i], latS[i])
                        mm(ssq, (ones_col), (sq), start=(i == 0), stop=(i == NLAT - 1))
                    sqrt_row = shp.tile([1, SW], F32, name="sqrt_row", tag="row1")
                    nc.scalar.activation(sqrt_row, ssq, AF.Sqrt, bias=eps_col[0:1, :], scale=1.0 / LORA)
                    scale_row = shp.tile([1, SW], F32R, name="scale_row", tag="row2")
                    with nc.allow_low_precision(reason="fp32r row for broadcast matmul"):
                        nc.vector.reciprocal(scale_row, sqrt_row)
                    bc = ppn.tile([128, SW], F32, name="bc", tag="bc")

                    def _latent_tail():
                        # deferred into P0b so the bc matmul (which waits on the
                        # reciprocal) doesn't stall queued PE work (FIFO queue)
                        mm(bc, (ones_row), (scale_row), start=True, stop=True)
                        latSB = [shp.tile([128, SW], BF16, name=f"lB{i}", tag=f"lB{i}") for i in range(NLAT)]
                        with nc.allow_low_precision(reason="bf16 latent feed for PE"):
                            for i in range(NLAT):
                                nc.vector.tensor_mul(latSB[i], latS[i], bc)
                        # ship the shard (gpsimd queue so the sync queue stays free)
                        for i in range(NLAT):
                            nc.gpsimd.dma_start(out=cc_in[ts(i, 128), :], in_=latSB[i])
                        nc.gpsimd.dma_start(out=cc_in[ds(LORA, ROPE), :], in_=kpeS)
                        nc.gpsimd.collective_compute(
                            "AllGather",
                            mybir.AluOpType.bypass,
                            replica_groups=[list(range(NCORES))],
                            ins=[cc_in[:, :].opt()],
                            outs=[cc_out[:, :].opt()],
                        )
                        # unpack gathered latent/kpe — gpsimd queue ONLY: any
                        # other queue would stall its later work behind the
                        # cc-completion wait (FIFO queues)
                        for c in range(NCORES):
                            cw = ds(c * SW, SW)
                            for i in range(NLAT):
                                nc.gpsimd.dma_start(out=latB[i][:, cw], in_=cc_out[ds(c * CKV + i * 128, 128), :])
                            nc.gpsimd.dma_start(out=kpeZ[0][0:64, cw], in_=cc_out[ds(c * CKV + LORA, ROPE), :])
                            nc.gpsimd.dma_start(out=kpeZ[1][64:128, cw], in_=cc_out[ds(c * CKV + LORA, ROPE), :])

                    # ---- P0b: q projections qT = Wq.T @ xT-chunks (overlaps AllGather) ----
                    with (
                        tc.tile_pool(name="x_pool", bufs=3) as xp,
                        tc.tile_pool(name="ps0", bufs=1, space="PSUM") as pp0,
                    ):
                        MS = [(0, 128), (128, 128), (256, 128)]  # h0 nope, h1 nope, ropes
                        for j in range(NJ):
                            jc = ds(j * LCH, LCH)
                            pss = [pp0.tile([128, LCH], F32, name=f"pm{m}", tag=f"pm{m}") for m in range(3)]
                            for k in range(NK):
                                xt = xp.tile([128, LCH], BF16, name="xt", tag="xt")
                                nc.sync.dma_start(out=xt, in_=xT[ts(k, 128), jc])
                                for m, (c0, cw) in enumerate(MS):
                                    mm(pss[m][:cw, :], (w_sb[k][:, ds(c0, cw)]), (xt),
                                       start=(k == 0), stop=(k == NK - 1))
                                if j == 0 and k == 2:
                                    _latent_tail()
                            nc.vector.tensor_copy(qn[0][:, jc], pss[0])
                            nc.vector.tensor_copy(qn[1][:, jc], pss[1])
                            nc.vector.tensor_copy(qr[:, jc], pss[2])

            # ---- P2: per-head k^T = We'.T @ latB, v = latB.T @ Wu' ----
            with (
                tc.tile_pool(name="wep", bufs=1) as wep,
                tc.tile_pool(name="ps2", bufs=2, space="PSUM") as pp2,
            ):
                # weight loads on the scalar queue: the sync queue still has
                # P0b's x-tile triggers ahead of it at this point
                we_sb = []
                for h in range(HPC):
                    row = []
                    for i in range(NLAT):
                        t = wep.tile([128, NOPE], BF16, name=f"we{h}{i}", tag=f"we{h}{i}")
                        nc.scalar.dma_start(out=t, in_=we[h, ts(i, 128), :])
                        row.append(t)
                    we_sb.append(row)
                wu_sb = []
                for i in range(NLAT):
                    t = wep.tile([128, HPC * VDIM], BF16, name=f"wu{i}", tag=f"wu{i}")
                    nc.scalar.dma_start(out=t, in_=wu[ts(i, 128), :])
                    wu_sb.append(t)
                for si in range(NS):
                    pv = pp2.tile([128, HPC * VDIM], F32, name="pv", tag="pv")
                    for i in range(NLAT):
                        mm(pv, (latB[i][:, ts(si, 128)]), (wu_sb[i]),
                           start=(i == 0), stop=(i == NLAT - 1))
                    nc.vector.tensor_copy(vsb[:, ds(si * HPC * VDIM, HPC * VDIM)], pv)
                for h in range(HPC):
                    for j in range(NJ):
                        jc = ds(j * LCH, LCH)
                        pk = pp2.tile([128, LCH], F32, name="pk", tag="pk")
                        for i in range(NLAT):
                            mm(pk, (we_sb[h][i]), (latB[i][:, jc]),
                               start=(i == 0), stop=(i == NLAT - 1))
                        nc.vector.tensor_copy(kT[h][:, jc], pk)

        # ---- P3: causal attention in transposed layout, per (head, l-chunk) ----
        # Diagonal-band s-tiles (d = si - 4j in 0..3) only touch l >= 128d in
        # the chunk, so their matmuls/exp run on the trimmed moving range and
        # only the leading 128-wide diagonal block needs the triangle mask.
        with (
            tc.tile_pool(name="wop", bufs=1) as wop,
            tc.tile_pool(name="epool", bufs=3) as epool,
            tc.tile_pool(name="rows", bufs=2) as rows,
            tc.tile_pool(name="esums", bufs=2) as esums,
            tc.tile_pool(name="ps3", bufs=1, space="PSUM") as pp3,
        ):
            tri = mask_sb[:, ds(384, 128)]  # tri[p, q] = 1 iff q >= p
            # The PE queue is FIFO, so the pbc broadcast matmul (which waits on
            # the ~3.3us DVE reciprocal) is deferred into the NEXT (h,j)'s
            # score-matmul stream — by the time it issues, the reciprocal is
            # long done and no PE stall occurs.
            pending_norm = None
            for h in range(HPC):
                for j in range(NJ):
                    jc = ds(j * LCH, LCH)
                    nsi = 4 * j + 4  # causal: s-tiles 0..4j+3 touch l-chunk j
                    po = pp3.tile([128, LCH], F32, name="po", tag="po", bufs=2)
                    esum = esums.tile([128, LCH], F32R, name="esum", tag="esum")
                    for si in range(nsi):
                        d = si - 4 * j
                        off = max(0, 128 * d)  # valid l starts here (trimmed)
                        w = LCH - off
                        sub = ds(j * LCH + off, w)
                        psub = ds(off, w)
                        ps = pp3.tile([128, LCH], F32, name="ps", tag="ps", bufs=2)
                        mm(ps[:, psub], (kT[h][:, ts(si, 128)]), (qn[h][:, sub]),
                           start=True, stop=False)
                        mm(ps[:, psub], (kpeZ[h][:, ts(si, 128)]), (qr[:, sub]),
                           start=False, stop=True)
                        e = epool.tile([128, LCH], BF16, name="e", tag="e")
                        nc.scalar.activation(e[:, psub], ps[:, psub], AF.Exp, scale=SCALE)
                        if d >= 0:  # mask the 128-wide diagonal block
                            nc.vector.tensor_mul(e[:, ds(off, 128)], e[:, ds(off, 128)], tri)
                        with nc.allow_low_precision(reason="fp32 esum of bf16 tiles"):
                            if si == 0:
                                nc.vector.tensor_copy(esum, e)
                            else:
                                nc.vector.tensor_add(esum[:, psub], esum[:, psub], e[:, psub])
                        mm(po[:, psub], (vsb[:, ds(si * HPC * VDIM + h * VDIM, VDIM)]), (e[:, psub]),
                           start=(si == 0), stop=(si == nsi - 1), skip_group_check=True)
                        if si == 3 and pending_norm is not None:
                            pending_norm()
                            pending_norm = None
                    pcs = pp3.tile([1, LCH], F32, name="pcs", tag="pcs", bufs=2)
                    mm(pcs, (ones_col), (esum), start=True, stop=True)
                    # reciprocal of the [1,512] denominator row on one DVE lane
                    # costs 3.3us (6.5 ns/elem/lane) and stalled the deferred
                    # broadcast matmul; bounce it through a [128,4] reshape so
                    # all 128 lanes split the work (~150 ns)
                    crow = rows.tile([1, LCH], F32, name="crow", tag="crow")
                    nc.scalar.copy(crow, pcs)
                    prsh = rows.tile([128, 4], F32, name="prsh", tag="prsh")
                    nc.gpsimd.dma_start(out=prsh, in_=crow)
                    rcp4 = rows.tile([128, 4], F32R, name="rcp4", tag="rcp4")
                    with nc.allow_low_precision(reason="fp32r row for broadcast matmul"):
                        nc.vector.reciprocal(rcp4, prsh)
                    rrow = rows.tile([1, LCH], F32R, name="rrow", tag="rrow")
                    nc.gpsimd.dma_start(out=rrow, in_=rcp4)

                    def _norm_tail(h=h, jc=jc, po=po, rrow=rrow):
                        pbc = pp3.tile([128, LCH], F32, name="pbc", tag="pbc", bufs=2)
                        mm(pbc, (ones_row), (rrow), start=True, stop=True)
                        bcs = epool.tile([128, LCH], F32, name="bcs", tag="bcs")
                        nc.vector.tensor_copy(bcs, pbc)
                        with nc.allow_low_precision(reason="bf16 outT feed for PE"):
                            nc.vector.tensor_mul(outT[h][:, jc], po, bcs)

                    pending_norm = _norm_tail
            pending_norm()
            pending_norm = None

            # ---- P4: partial o_proj y = outT.T @ Wo[2-head rows] ----
            wo_sb = []
            for hh, wsrc in enumerate([wo0, wo1]):
                t = wop.tile([128, HID], BF16, name=f"wo{hh}", tag=f"wo{hh}")
                nc.scalar.dma_start(out=t, in_=wsrc[:, :])
                wo_sb.append(t)
            for i in range(NS):
                for n in range(NJ):
                    py = pp3.tile([128, LCH], F32, name="py", tag="ps", bufs=2)
                    mm(py, (outT[0][:, ts(i, 128)]), (wo_sb[0][:, ds(n * LCH, LCH)]),
                       start=True, stop=False)
                    mm(py, (outT[1][:, ts(i, 128)]), (wo_sb[1][:, ds(n * LCH, LCH)]),
                       start=False, stop=True)
                    ysb = epool.tile([128, LCH], BF16, name="ysb", tag="ysb", bufs=3)
                    if (i * NJ + n) % 2 == 0:  # split PSUM->SBUF casts across engines
                        nc.scalar.copy(ysb, py)
                    else:
                        nc.vector.tensor_copy(ysb, py)
                    nc.sync.dma_start(out=y[ts(i, 128), ds(n * LCH, LCH)], in_=ysb)

    _split_excess_waits(nc)
    return nc


_NC_CACHE = None


def _get_nc():
    global _NC_CACHE
    if _NC_CACHE is None:
        _NC_CACHE = _build_nc()
    return _NC_CACHE


def _make_in_maps(x, Wq, Wkv_a, kv_ln_w, W_embed, W_unembed, Wo):
    xT = np.ascontiguousarray(np.asarray(x, dtype=np.float32)[0].T).astype(BF)
    Wq = np.asarray(Wq, dtype=np.float32)
    Wkv_a = np.asarray(Wkv_a, dtype=np.float32)
    kv_ln_w = np.asarray(kv_ln_w, dtype=np.float32)
    W_embed = np.asarray(W_embed, dtype=np.float32)
    W_unembed = np.asarray(W_unembed, dtype=np.float32)
    Wo = np.asarray(Wo, dtype=np.float32)

    Wq3 = Wq.reshape(HID, H, QDIM)
    # diagonal-band mask template: mbig[p, q] = 1 iff (q - 384) >= p
    q_idx = np.arange(896) - 384
    p_idx = np.arange(128)
    mbig = (q_idx[None, :] >= p_idx[:, None]).astype(BF)

    in_maps = []
    for c in range(NCORES):
        h0, h1 = HPC * c, HPC * c + 1
        wqkv = np.concatenate(
            [
                Wq3[:, h0, :NOPE],
                Wq3[:, h1, :NOPE],
                Wq3[:, h0, NOPE:],
                Wq3[:, h1, NOPE:],
                Wkv_a,
                np.zeros((HID, WCOLS - 384 - Wkv_a.shape[1]), np.float32),
            ],
            axis=1,
        )
        we = np.ascontiguousarray(W_embed[[h0, h1]] * kv_ln_w[None, :, None])
        wu = np.ascontiguousarray(
            np.concatenate([W_unembed[h0].T, W_unembed[h1].T], axis=1) * kv_ln_w[:, None]
        )
        in_maps.append(
            {
                "xT": xT,
                "xTs": np.ascontiguousarray(
                    xT[:, c * SW : (c + 1) * SW].reshape(NK, 128, SW).transpose(1, 0, 2).reshape(128, NK * SW)
                ),
                "wqkv": np.ascontiguousarray(wqkv).astype(BF),
                "we": we.astype(BF),
                "wu": wu.astype(BF),
                "wo0": np.ascontiguousarray(Wo[h0 * VDIM : (h0 + 1) * VDIM]).astype(BF),
                "wo1": np.ascontiguousarray(Wo[h1 * VDIM : (h1 + 1) * VDIM]).astype(BF),
                "mbig": mbig,
                "ones_col_d": np.ones((128, 1), np.float32),
                "ones_row_d": np.ones((1, 128), np.float32),
            }
        )
    return in_maps


def run(trace=False, tmpdir=None, **inputs):
    """Run the SPMD kernel; returns (full_output, BassKernelResults)."""
    inputs.pop("mask", None)  # causal structure is hardcoded
    nc = _get_nc()
    in_maps = _make_in_maps(**inputs)
    res = run_bass_kernel_spmd(
        nc, in_maps, core_ids=list(range(NCORES)), trace=trace, tmpdir=tmpdir
    )
    y = np.zeros((L, HID), dtype=np.float32)
    for c in range(NCORES):
        y += np.asarray(res.results[c]["y"], dtype=np.float32)
    return y.reshape(B, L, HID), res


def kernel(**inputs):
    y, _ = run(trace=False, **inputs)
    return y
